# revision 4
# baseline (speedup 1.0000x reference)
"""Biaffine span classifier kernel for 8 Trainium2 NeuronCores.

Math (per batch b, label o):
    start = relu(x @ W_start + b_start); end = relu(x @ W_end + b_end)
    rotate both with tiled-halves sinusoidal tables
    span[o,x,y] = startR[x,:] @ weight[o] @ endR[y,:]^T
    span = span*pad[y] - (1-pad[y])*NEG - NEG*tril(x>y)

Sharding: core c = b*2 + half handles batch b and labels [half*8, half*8+8).
Each core writes a contiguous [8, S, S] slice of the output.

On-chip layout is transposed ([H, S], H on partitions) so all contractions
run on the PE with the contraction dim on partitions. The mask's additive
term is folded into the big matmul via an augmented K=65 contraction
(ones row in tmpT, add_row in endT). Blocks entirely below the diagonal
are exactly -NEG in fp32 (|span| << 0.5*ulp(NEG)), so they skip the
matmul and copy a constant band instead.
"""

import numpy as np

B, S, I, H, O = 4, 1024, 768, 64, 16
NCORES = 8
OH = O // NCORES * 4  # 8 labels per core (O split in halves of 8)
NEG = 1.0e12
KT = I // 128  # 6 k-tiles over the input dim
ST = S // 128  # 8 s-tiles

_STATE = {}


def _tables():
    """Host-precomputed constants (mimic reference fp32 ops)."""
    position = np.arange(S, dtype=np.float32)
    idx = np.arange(H // 2, dtype=np.float32)
    expo = (np.float32(-2.0) * idx) / np.float32(H)
    inv_freq = np.power(np.float32(10000.0), expo).astype(np.float32)
    ang = position[:, None] * inv_freq[None, :]          # [S, 32] f32
    cos_h = np.cos(ang).astype(np.float32).T             # [32, S]
    sin_h = np.sin(ang).astype(np.float32).T
    cosT = np.ascontiguousarray(np.concatenate([cos_h, cos_h], axis=0))  # [64, S]
    sinT = np.ascontiguousarray(np.concatenate([sin_h, sin_h], axis=0))
    # swap matrix as lhsT: out[2m] = -in[2m+1]; out[2m+1] = in[2m]
    msw = np.zeros((H, H), np.float32)
    for m in range(H // 2):
        msw[2 * m + 1, 2 * m] = -1.0
        msw[2 * m, 2 * m + 1] = 1.0
    # tril patterns for diagonal-crossing blocks: P_k[x', y'] = -NEG if x'+128k > y'
    xp = np.arange(128, dtype=np.int64)[:, None]
    yp = np.arange(512, dtype=np.int64)[None, :]
    tril = np.stack(
        [np.where(xp + 128 * k > yp, np.float32(-NEG), np.float32(0.0))
         for k in range(4)]
    ).astype(np.float32)                                  # [4, 128, 512]
    ident = np.eye(128, dtype=np.float32)
    return cosT, sinT, msw, tril, ident


def _build():
    import concourse.bacc as bacc
    import concourse.bass as bass
    import concourse.mybir as mybir
    from concourse import tile

    f32 = mybir.dt.float32
    AF = mybir.ActivationFunctionType
    ALU = mybir.AluOpType
    PSUM = bass.MemorySpace.PSUM

    nc = bacc.Bacc("TRN2", target_bir_lowering=False, debug=False,
                   num_devices=NCORES)

    x_t = nc.dram_tensor("x", [S, I], f32, kind="ExternalInput")
    mask_t = nc.dram_tensor("mask", [1, S], f32, kind="ExternalInput")
    ws_t = nc.dram_tensor("w_start", [I, H], f32, kind="ExternalInput")
    we_t = nc.dram_tensor("w_end", [I, H], f32, kind="ExternalInput")
    bs_t = nc.dram_tensor("b_start", [H, 1], f32, kind="ExternalInput")
    be_t = nc.dram_tensor("b_end", [H, 1], f32, kind="ExternalInput")
    wo_t = nc.dram_tensor("w_o", [OH, H, H], f32, kind="ExternalInput")
    cos_t = nc.dram_tensor("cos_t", [H, S], f32, kind="ExternalInput")
    sin_t = nc.dram_tensor("sin_t", [H, S], f32, kind="ExternalInput")
    msw_t = nc.dram_tensor("mswap", [H, H], f32, kind="ExternalInput")
    tril_t = nc.dram_tensor("trilneg", [4, 128, 512], f32, kind="ExternalInput")
    id_t = nc.dram_tensor("ident", [128, 128], f32, kind="ExternalInput")
    out_t = nc.dram_tensor("out", [OH, S, S], f32, kind="ExternalOutput")

    out_r = out_t.ap().rearrange("o (xb p) y -> o p xb y", p=128)

    with tile.TileContext(nc) as tc:
        with tc.tile_pool(name="persist", bufs=1) as pp, \
             tc.tile_pool(name="scratch", bufs=2) as sp:
            # ---- constants and weights ----
            ident = pp.tile([128, 128], f32)
            nc.sync.dma_start(ident[:], id_t.ap())
            cosT = pp.tile([H, S], f32)
            nc.sync.dma_start(cosT[:], cos_t.ap())
            sinT = pp.tile([H, S], f32)
            nc.sync.dma_start(sinT[:], sin_t.ap())
            mswap = pp.tile([H, H], f32)
            nc.sync.dma_start(mswap[:], msw_t.ap())
            tril = pp.tile([128, 4, 512], f32)
            nc.sync.dma_start(tril[:], tril_t.ap().rearrange("k p y -> p k y"))
            wsT = pp.tile([128, KT, H], f32)
            nc.sync.dma_start(wsT[:], ws_t.ap().rearrange("(t p) h -> p t h", p=128))
            weT = pp.tile([128, KT, H], f32)
            nc.sync.dma_start(weT[:], we_t.ap().rearrange("(t p) h -> p t h", p=128))
            bsv = pp.tile([H, 1], f32)
            nc.sync.dma_start(bsv[:], bs_t.ap())
            bev = pp.tile([H, 1], f32)
            nc.sync.dma_start(bev[:], be_t.ap())
            wo = pp.tile([H, OH, H], f32)
            nc.sync.dma_start(wo[:], wo_t.ap().rearrange("o i j -> i o j"))
            mask0 = pp.tile([1, S], f32)
            nc.sync.dma_start(mask0[:], mask_t.ap())
            ones1 = pp.tile([1, 128], f32)
            nc.gpsimd.memset(ones1[:], 1.0)

            # ---- transpose x: xa [s-part, i] -> xT [i-part, s] via PE ----
            xT = pp.tile([128, KT, S], f32)
            with tc.tile_pool(name="xprep", bufs=1) as xp_pool, \
                 tc.tile_pool(name="ps_tr", bufs=4, space=PSUM) as ptr:
                xa = xp_pool.tile([128, ST, I], f32)
                nc.sync.dma_start(
                    xa[:], x_t.ap().rearrange("(t p) i -> p t i", p=128))
                for sb in range(ST):
                    for kb in range(KT):
                        ps_tr = ptr.tile([128, 128], f32, name="ps_tr")
                        nc.tensor.transpose(
                            ps_tr[:], xa[:, sb, kb * 128:(kb + 1) * 128],
                            ident[:])
                        dst = xT[:, kb, sb * 128:(sb + 1) * 128]
                        if (sb * KT + kb) % 2 == 0:
                            nc.vector.tensor_copy(dst, ps_tr[:])
                        else:
                            nc.scalar.copy(dst, ps_tr[:])

            # ---- mask-derived vectors ----
            addrow0 = pp.tile([1, S], f32)
            nc.vector.tensor_scalar(
                addrow0[:], mask0[:], float(NEG), float(-NEG),
                ALU.mult, ALU.add)                     # (pad-1)*NEG as pad*NEG-NEG

            # ---- projections + rotation ----
            startR = pp.tile([H, S], f32)
            endA = pp.tile([H + 1, S], f32)           # rows 0..63 endR*pad, row 64 addrow
            padB = pp.tile([H, S], f32)
            constband = pp.tile([128, 512], f32)      # -NEG + addrow[y], y in [0,512)

            with tc.tile_pool(name="ps_proj", bufs=2, space=PSUM) as ppj:
                # pad broadcast [64, S] and constband [128, 512) via K=1 matmuls
                for h in range(2):
                    ps_pb = ppj.tile([H, 512], f32, name="ps_pb")
                    nc.tensor.matmul(ps_pb[:], ones1[:, :H],
                                     mask0[:, h * 512:(h + 1) * 512],
                                     start=True, stop=True)
                    nc.scalar.copy(padB[:, h * 512:(h + 1) * 512], ps_pb[:])
                ps_cb = ppj.tile([128, 512], f32, name="ps_cb", bufs=1)
                nc.tensor.matmul(ps_cb[:], ones1[:], addrow0[:, 0:512],
                                 start=True, stop=True)
                nc.scalar.activation(constband[:], ps_cb[:], AF.Copy,
                                     bias=float(-NEG))

                for side in range(2):                 # 0 = start, 1 = end
                    wT = wsT if side == 0 else weT
                    bv = bsv if side == 0 else bev
                    for h in range(2):
                        sl = slice(h * 512, (h + 1) * 512)
                        ps_proj = ppj.tile([H, 512], f32, name="ps_proj")
                        for kb in range(KT):
                            nc.tensor.matmul(
                                ps_proj[:], wT[:, kb, :], xT[:, kb, sl],
                                start=(kb == 0), stop=(kb == KT - 1))
                        relu = sp.tile([H, 512], f32, name="relu")
                        nc.scalar.activation(relu[:], ps_proj[:], AF.Relu,
                                             bias=bv[:])
                        ps_swap = ppj.tile([H, 512], f32, name="ps_swap")
                        nc.tensor.matmul(ps_swap[:], mswap[:], relu[:],
                                         start=True, stop=True)
                        rm = sp.tile([H, 512], f32, name="rm")
                        nc.vector.tensor_mul(rm[:], relu[:], cosT[:, sl])
                        rs = sp.tile([H, 512], f32, name="rs")
                        nc.vector.tensor_mul(rs[:], ps_swap[:], sinT[:, sl])
                        if side == 0:
                            nc.vector.tensor_add(startR[:, sl], rm[:], rs[:])
                        else:
                            es = sp.tile([H, 512], f32, name="es")
                            nc.vector.tensor_add(es[:], rm[:], rs[:])
                            nc.vector.tensor_mul(endA[0:H, sl], es[:],
                                                 padB[:, sl])
            # row 64 of endA = addrow (cross-partition move via DMA)
            nc.sync.dma_start(endA[H:H + 1, :], addrow0[:])

            # ---- staging buffers; pre-fill constant (all-below-diagonal) blocks ----
            stgA = pp.tile([128, ST, S], f32)
            stgB = pp.tile([128, ST, S], f32)
            tmpA0 = pp.tile([H + 1, S], f32)
            tmpA1 = pp.tile([H + 1, S], f32)
            nc.gpsimd.memset(tmpA0[H:H + 1, :], 1.0)
            nc.gpsimd.memset(tmpA1[H:H + 1, :], 1.0)
            for stg in (stgA, stgB):
                for xb in range(4, 8):
                    nc.gpsimd.tensor_copy(stg[:, xb, 0:512], constband[:])

            # ---- main loop over labels ----
            with tc.tile_pool(name="ps_main", bufs=2, space=PSUM) as pm, \
                 tc.tile_pool(name="ps_span_pool", bufs=4, space=PSUM) as pspan:
                for o in range(OH):
                    stg = stgA if o % 2 == 0 else stgB
                    tmpA = tmpA0 if o % 2 == 0 else tmpA1
                    # tmpT[j, x] = sum_i weight[o][i, j] * startR[i, x]
                    for h in range(2):
                        sl = slice(h * 512, (h + 1) * 512)
                        ps_tmp = pm.tile([H, 512], f32, name="ps_tmp")
                        nc.tensor.matmul(ps_tmp[:],
                                         wo[:, o, :], startR[:, sl],
                                         start=True, stop=True)
                        nc.scalar.copy(tmpA[0:H, sl], ps_tmp[:])
                    # span blocks
                    nplain = 0
                    for xb in range(ST):
                        lhs = tmpA[:, xb * 128:(xb + 1) * 128]
                        # yb = 1 always computed
                        ps_sp = pspan.tile([128, 512], f32, name="ps_sp",
                                           tag="ps_sp")
                        nc.tensor.matmul(ps_sp[:], lhs, endA[:, 512:1024],
                                         start=True, stop=True)
                        dst = stg[:, xb, 512:1024]
                        if xb >= 4:
                            nc.vector.tensor_tensor(
                                dst, ps_sp[:], tril[:, xb - 4, :], ALU.add)
                        else:
                            if nplain % 2 == 0:
                                nc.vector.tensor_copy(dst, ps_sp[:])
                            else:
                                nc.scalar.copy(dst, ps_sp[:])
                            nplain += 1
                        # yb = 0 computed only for xb < 4 (else constant band)
                        if xb < 4:
                            ps_sp2 = pspan.tile([128, 512], f32, name="ps_sp2",
                                                tag="ps_sp")
                            nc.tensor.matmul(ps_sp2[:], lhs, endA[:, 0:512],
                                             start=True, stop=True)
                            nc.vector.tensor_tensor(
                                stg[:, xb, 0:512], ps_sp2[:],
                                tril[:, xb, :], ALU.add)
                    nc.sync.dma_start(out_r[o], stg[:])

    nc.compile()
    return nc


def _get_nc():
    if "nc" not in _STATE:
        _STATE["nc"] = _build()
    return _STATE["nc"]


def _make_in_maps(x, mask, W_start, b_start, W_end, b_end, weight):
    cosT, sinT, msw, tril, ident = _tables()
    x = np.ascontiguousarray(np.asarray(x, np.float32))
    mask = np.ascontiguousarray(np.asarray(mask, np.float32))
    W_start = np.ascontiguousarray(np.asarray(W_start, np.float32))
    W_end = np.ascontiguousarray(np.asarray(W_end, np.float32))
    b_start = np.ascontiguousarray(np.asarray(b_start, np.float32).reshape(H, 1))
    b_end = np.ascontiguousarray(np.asarray(b_end, np.float32).reshape(H, 1))
    weight = np.ascontiguousarray(np.asarray(weight, np.float32))
    in_maps = []
    for c in range(NCORES):
        b, half = c // 2, c % 2
        in_maps.append({
            "x": np.ascontiguousarray(x[b]),
            "mask": np.ascontiguousarray(mask[b:b + 1]),
            "w_start": W_start,
            "w_end": W_end,
            "b_start": b_start,
            "b_end": b_end,
            "w_o": np.ascontiguousarray(weight[half * OH:(half + 1) * OH]),
            "cos_t": cosT,
            "sin_t": sinT,
            "mswap": msw,
            "trilneg": tril,
            "ident": ident,
        })
    return in_maps


def _execute(in_maps, trace=False):
    from concourse.bass_utils import run_bass_kernel_spmd
    nc = _get_nc()
    return run_bass_kernel_spmd(nc, in_maps, list(range(NCORES)), trace=trace)


def kernel(x, mask, W_start, b_start, W_end, b_end, weight):
    in_maps = _make_in_maps(x, mask, W_start, b_start, W_end, b_end, weight)
    res = _execute(in_maps)
    outs = [res.results[c]["out"] for c in range(NCORES)]
    full = np.stack(outs).reshape(B, 2, OH, S, S).reshape(B, O, S, S)
    return full.astype(np.float32)


# revision 7
# speedup vs baseline: 1.8655x; 1.8655x over previous
"""Biaffine span classifier kernel for 8 Trainium2 NeuronCores.

Math (per batch b, label o):
    start = relu(x @ W_start + b_start); end = relu(x @ W_end + b_end)
    rotate both with tiled-halves sinusoidal tables
    span[o,x,y] = startR[x,:] @ weight[o] @ endR[y,:]^T
    span = span*pad[y] - (1-pad[y])*NEG - NEG*tril(x>y)

Sharding: core c = b*2 + half handles batch b and labels [half*8, half*8+8).
Each core writes a contiguous [8, S, S] slice of the output.

On-chip layout is transposed ([H, S], H on partitions); x is transposed on
the host so every contraction has its reduction dim on partitions. All
matmuls run in fp32r mode (single-pass PE, ~TF32 precision) — operands are
rounded to fp32r by their producers as the BIR verifier requires. The mask's
additive term is folded into the big matmul via an augmented K=65
contraction (ones row in tmpT, add_row in endT). Blocks entirely below the
diagonal are exactly -NEG in fp32 (|span| << 0.5*ulp(NEG)), so they skip
the matmul and copy a constant band instead.
"""

import numpy as np

B, S, I, H, O = 4, 1024, 768, 64, 16
NCORES = 8
OH = O // 2  # 8 labels per core
NEG = 1.0e12
KT = I // 128  # 6 k-tiles over the input dim
ST = S // 128  # 8 s-tiles

_STATE = {}


def _tables():
    """Host-precomputed constants (mimic reference fp32 ops)."""
    position = np.arange(S, dtype=np.float32)
    idx = np.arange(H // 2, dtype=np.float32)
    expo = (np.float32(-2.0) * idx) / np.float32(H)
    inv_freq = np.power(np.float32(10000.0), expo).astype(np.float32)
    ang = position[:, None] * inv_freq[None, :]          # [S, 32] f32
    cos_h = np.cos(ang).astype(np.float32).T             # [32, S]
    sin_h = np.sin(ang).astype(np.float32).T
    cosT = np.ascontiguousarray(np.concatenate([cos_h, cos_h], axis=0))  # [64, S]
    sinT = np.ascontiguousarray(np.concatenate([sin_h, sin_h], axis=0))
    # swap matrix as lhsT: out[2m] = -in[2m+1]; out[2m+1] = in[2m]
    msw = np.zeros((H, H), np.float32)
    for m in range(H // 2):
        msw[2 * m + 1, 2 * m] = -1.0
        msw[2 * m, 2 * m + 1] = 1.0
    # tril patterns for diagonal-crossing blocks: P_k[x', y'] = -NEG if x'+128k > y'
    xp = np.arange(128, dtype=np.int64)[:, None]
    yp = np.arange(512, dtype=np.int64)[None, :]
    tril = np.stack(
        [np.where(xp + 128 * k > yp, np.float32(-NEG), np.float32(0.0))
         for k in range(4)]
    ).astype(np.float32)                                  # [4, 128, 512]
    return cosT, sinT, msw, tril


def _build():
    import concourse.bacc as bacc
    import concourse.bass as bass
    import concourse.mybir as mybir
    from concourse import tile

    f32 = mybir.dt.float32
    f32r = mybir.dt.float32r
    AF = mybir.ActivationFunctionType
    ALU = mybir.AluOpType
    PSUM = bass.MemorySpace.PSUM

    nc = bacc.Bacc("TRN2", target_bir_lowering=False, debug=False,
                   num_devices=NCORES)

    xT_t = nc.dram_tensor("xT", [I, S], f32, kind="ExternalInput")
    mask_t = nc.dram_tensor("mask", [1, S], f32, kind="ExternalInput")
    ws_t = nc.dram_tensor("w_start", [I, H], f32, kind="ExternalInput")
    we_t = nc.dram_tensor("w_end", [I, H], f32, kind="ExternalInput")
    bs_t = nc.dram_tensor("b_start", [H, 1], f32, kind="ExternalInput")
    be_t = nc.dram_tensor("b_end", [H, 1], f32, kind="ExternalInput")
    wo_t = nc.dram_tensor("w_o", [OH, H, H], f32, kind="ExternalInput")
    cos_t = nc.dram_tensor("cos_t", [H, S], f32, kind="ExternalInput")
    sin_t = nc.dram_tensor("sin_t", [H, S], f32, kind="ExternalInput")
    msw_t = nc.dram_tensor("mswap", [H, H], f32, kind="ExternalInput")
    tril_t = nc.dram_tensor("trilneg", [4, 128, 512], f32, kind="ExternalInput")
    out_t = nc.dram_tensor("out", [OH, S, S], f32, kind="ExternalOutput")

    out_r = out_t.ap().rearrange("o (xb p) y -> o p xb y", p=128)

    def r(ap):
        return ap.bitcast(f32r)

    with tile.TileContext(nc) as tc:
        with tc.tile_pool(name="persist", bufs=1) as pp, \
             tc.tile_pool(name="scratch", bufs=2) as sp:
            # ---- constants ----
            cosT = pp.tile([H, S], f32)
            nc.sync.dma_start(cosT[:], cos_t.ap())
            sinT = pp.tile([H, S], f32)
            nc.sync.dma_start(sinT[:], sin_t.ap())
            tril = pp.tile([128, 4, 512], f32)
            nc.sync.dma_start(tril[:], tril_t.ap().rearrange("k p y -> p k y"))
            bsv = pp.tile([H, 1], f32)
            nc.sync.dma_start(bsv[:], bs_t.ap())
            bev = pp.tile([H, 1], f32)
            nc.sync.dma_start(bev[:], be_t.ap())
            mask0 = pp.tile([1, S], f32)
            nc.sync.dma_start(mask0[:], mask_t.ap())
            ones1f = pp.tile([1, 128], f32)
            nc.gpsimd.memset(ones1f[:], 1.0)
            ones1 = pp.tile([1, 128], f32)
            nc.vector.tensor_copy(r(ones1[:]), ones1f[:])
            onesrow = pp.tile([1, S], f32)
            nc.gpsimd.memset(onesrow[:], 1.0)
            onesrowr = pp.tile([1, S], f32)
            nc.vector.tensor_copy(r(onesrowr[:]), onesrow[:])

            # matmul operands: DMA f32 then round to f32r on-chip
            wsT = pp.tile([128, KT, H], f32)
            weT = pp.tile([128, KT, H], f32)
            wo = pp.tile([H, OH, H], f32)
            mswap = pp.tile([H, H], f32)
            mask0r = pp.tile([1, S], f32)
            xTr = pp.tile([128, KT, S], f32)
            with tc.tile_pool(name="load", bufs=1) as lp:
                wsL = lp.tile([128, KT, H], f32)
                nc.sync.dma_start(
                    wsL[:], ws_t.ap().rearrange("(t p) h -> p t h", p=128))
                nc.vector.tensor_copy(r(wsT[:]), wsL[:])
                weL = lp.tile([128, KT, H], f32)
                nc.sync.dma_start(
                    weL[:], we_t.ap().rearrange("(t p) h -> p t h", p=128))
                nc.vector.tensor_copy(r(weT[:]), weL[:])
                woL = lp.tile([H, OH, H], f32)
                nc.sync.dma_start(woL[:], wo_t.ap().rearrange("o i j -> i o j"))
                nc.vector.tensor_copy(r(wo[:]), woL[:])
                mswL = lp.tile([H, H], f32)
                nc.sync.dma_start(mswL[:], msw_t.ap())
                nc.vector.tensor_copy(r(mswap[:]), mswL[:])
                nc.vector.tensor_copy(r(mask0r[:]), mask0[:])
                xTin = lp.tile([128, KT, S], f32)
                nc.sync.dma_start(
                    xTin[:], xT_t.ap().rearrange("(t p) s -> p t s", p=128))
                for t in range(KT):
                    eng = nc.vector if t % 2 == 0 else nc.scalar
                    if t % 2 == 0:
                        nc.vector.tensor_copy(r(xTr[:, t, :]), xTin[:, t, :])
                    else:
                        nc.scalar.copy(r(xTr[:, t, :]), xTin[:, t, :])

                # ---- mask-derived vectors ----
                addrow0 = pp.tile([1, S], f32)
                nc.vector.tensor_scalar(
                    r(addrow0[:]), mask0[:], float(NEG), float(-NEG),
                    ALU.mult, ALU.add)                 # (pad-1)*NEG

                # ---- projections + rotation ----
                startR = pp.tile([H, S], f32)
                endA = pp.tile([H + 1, S], f32)       # 0..63 endR*pad, 64 addrow
                padB = pp.tile([H, S], f32)
                constband = pp.tile([128, 512], f32)  # -NEG + addrow[y], y<512

                with tc.tile_pool(name="ps_proj", bufs=2, space=PSUM) as ppj:
                    for h in range(2):
                        sl = slice(h * 512, (h + 1) * 512)
                        ps_pb = ppj.tile([H, 512], f32, name="ps_pb")
                        nc.tensor.matmul(ps_pb[:], r(ones1[:, :H]),
                                         r(mask0r[:, sl]),
                                         start=True, stop=True)
                        nc.scalar.copy(padB[:, sl], ps_pb[:])
                    ps_cb = ppj.tile([128, 512], f32, name="ps_cb", bufs=1)
                    nc.tensor.matmul(ps_cb[:], r(ones1[:]), r(addrow0[:, 0:512]),
                                     start=True, stop=True)
                    nc.scalar.activation(constband[:], ps_cb[:], AF.Copy,
                                         bias=float(-NEG))

                    for side in range(2):             # 0 = start, 1 = end
                        wT = wsT if side == 0 else weT
                        bv = bsv if side == 0 else bev
                        for h in range(2):
                            sl = slice(h * 512, (h + 1) * 512)
                            ps_proj = ppj.tile([H, 512], f32, name="ps_proj")
                            for kb in range(KT):
                                nc.tensor.matmul(
                                    ps_proj[:], r(wT[:, kb, :]),
                                    r(xTr[:, kb, sl]),
                                    start=(kb == 0), stop=(kb == KT - 1))
                            relu = sp.tile([H, 512], f32, name="relu")
                            nc.scalar.activation(r(relu[:]), ps_proj[:],
                                                 AF.Relu, bias=bv[:])
                            ps_swap = ppj.tile([H, 512], f32, name="ps_swap")
                            nc.tensor.matmul(ps_swap[:], r(mswap[:]), r(relu[:]),
                                             start=True, stop=True)
                            rm = sp.tile([H, 512], f32, name="rm")
                            nc.vector.tensor_mul(rm[:], relu[:], cosT[:, sl])
                            rs = sp.tile([H, 512], f32, name="rs")
                            nc.vector.tensor_mul(rs[:], ps_swap[:], sinT[:, sl])
                            if side == 0:
                                nc.vector.tensor_add(r(startR[:, sl]),
                                                     rm[:], rs[:])
                            else:
                                es = sp.tile([H, 512], f32, name="es")
                                nc.vector.tensor_add(es[:], rm[:], rs[:])
                                nc.vector.tensor_mul(r(endA[0:H, sl]), es[:],
                                                     padB[:, sl])
                # row 64 of endA = addrow (cross-partition move via DMA)
                nc.sync.dma_start(r(endA[H:H + 1, :]), r(addrow0[:]))

            # ---- staging buffers; pre-fill constant blocks ----
            stgA = pp.tile([128, ST, S], f32)
            stgB = pp.tile([128, ST, S], f32)
            tmpA0 = pp.tile([H + 1, S], f32)
            tmpA1 = pp.tile([H + 1, S], f32)
            nc.sync.dma_start(r(tmpA0[H:H + 1, :]), r(onesrowr[:]))
            nc.sync.dma_start(r(tmpA1[H:H + 1, :]), r(onesrowr[:]))
            for stg in (stgA, stgB):
                for xb in range(4, 8):
                    nc.gpsimd.tensor_copy(stg[:, xb, 0:512], constband[:])

            # ---- main loop over labels ----
            with tc.tile_pool(name="ps_main", bufs=2, space=PSUM) as pm, \
                 tc.tile_pool(name="ps_span_pool", bufs=4, space=PSUM) as pspan:
                for o in range(OH):
                    stg = stgA if o % 2 == 0 else stgB
                    tmpA = tmpA0 if o % 2 == 0 else tmpA1
                    # tmpT[j, x] = sum_i weight[o][i, j] * startR[i, x]
                    for h in range(2):
                        sl = slice(h * 512, (h + 1) * 512)
                        ps_tmp = pm.tile([H, 512], f32, name="ps_tmp")
                        nc.tensor.matmul(ps_tmp[:],
                                         r(wo[:, o, :]), r(startR[:, sl]),
                                         start=True, stop=True)
                        nc.scalar.copy(r(tmpA[0:H, sl]), ps_tmp[:])
                    # span blocks
                    nplain = 0
                    for xb in range(ST):
                        lhs = r(tmpA[:, xb * 128:(xb + 1) * 128])
                        # yb = 1 always computed
                        ps_sp = pspan.tile([128, 512], f32, name="ps_sp",
                                           tag="ps_sp")
                        nc.tensor.matmul(ps_sp[:], lhs, r(endA[:, 512:1024]),
                                         start=True, stop=True)
                        dst = stg[:, xb, 512:1024]
                        if xb >= 4:
                            nc.vector.tensor_tensor(
                                dst, ps_sp[:], tril[:, xb - 4, :], ALU.add)
                        else:
                            if nplain % 2 == 0:
                                nc.vector.tensor_copy(dst, ps_sp[:])
                            else:
                                nc.scalar.copy(dst, ps_sp[:])
                            nplain += 1
                        # yb = 0 computed only for xb < 4 (else constant band)
                        if xb < 4:
                            ps_sp2 = pspan.tile([128, 512], f32, name="ps_sp2",
                                                tag="ps_sp")
                            nc.tensor.matmul(ps_sp2[:], lhs, r(endA[:, 0:512]),
                                             start=True, stop=True)
                            nc.vector.tensor_tensor(
                                stg[:, xb, 0:512], ps_sp2[:],
                                tril[:, xb, :], ALU.add)
                    nc.sync.dma_start(out_r[o], stg[:])

    nc.compile()
    return nc


def _get_nc():
    if "nc" not in _STATE:
        _STATE["nc"] = _build()
    return _STATE["nc"]


def _make_in_maps(x, mask, W_start, b_start, W_end, b_end, weight):
    cosT, sinT, msw, tril = _tables()
    x = np.asarray(x, np.float32)
    mask = np.ascontiguousarray(np.asarray(mask, np.float32))
    W_start = np.ascontiguousarray(np.asarray(W_start, np.float32))
    W_end = np.ascontiguousarray(np.asarray(W_end, np.float32))
    b_start = np.ascontiguousarray(np.asarray(b_start, np.float32).reshape(H, 1))
    b_end = np.ascontiguousarray(np.asarray(b_end, np.float32).reshape(H, 1))
    weight = np.ascontiguousarray(np.asarray(weight, np.float32))
    in_maps = []
    for c in range(NCORES):
        b, half = c // 2, c % 2
        in_maps.append({
            "xT": np.ascontiguousarray(x[b].T),
            "mask": np.ascontiguousarray(mask[b:b + 1]),
            "w_start": W_start,
            "w_end": W_end,
            "b_start": b_start,
            "b_end": b_end,
            "w_o": np.ascontiguousarray(weight[half * OH:(half + 1) * OH]),
            "cos_t": cosT,
            "sin_t": sinT,
            "mswap": msw,
            "trilneg": tril,
        })
    return in_maps


def _execute(in_maps, trace=False):
    from concourse.bass_utils import run_bass_kernel_spmd
    nc = _get_nc()
    return run_bass_kernel_spmd(nc, in_maps, list(range(NCORES)), trace=trace)


def kernel(x, mask, W_start, b_start, W_end, b_end, weight):
    in_maps = _make_in_maps(x, mask, W_start, b_start, W_end, b_end, weight)
    res = _execute(in_maps)
    outs = [res.results[c]["out"] for c in range(NCORES)]
    full = np.stack(outs).reshape(B, 2, OH, S, S).reshape(B, O, S, S)
    return full.astype(np.float32)


# revision 8
# speedup vs baseline: 2.0960x; 1.1236x over previous
"""Biaffine span classifier kernel for 8 Trainium2 NeuronCores.

Math (per batch b, label o):
    start = relu(x @ W_start + b_start); end = relu(x @ W_end + b_end)
    rotate both with tiled-halves sinusoidal tables
    span[o,x,y] = startR[x,:] @ weight[o] @ endR[y,:]^T
    span = span*pad[y] - (1-pad[y])*NEG - NEG*tril(x>y)

Sharding: core c = b*2 + half handles batch b and labels [half*8, half*8+8).
Each core writes a contiguous [8, S, S] slice of the output.

On-chip layout is transposed ([H, S], H on partitions); x is transposed on
the host so every contraction has its reduction dim on partitions. All
matmuls run in fp32r mode (single-pass PE, ~TF32 precision) — operands are
rounded to fp32r by their producers as the BIR verifier requires. The mask's
additive term is folded into the big matmul via an augmented K=65
contraction (ones row in tmpT, add_row in endT). Blocks entirely below the
diagonal are exactly -NEG in fp32 (|span| << 0.5*ulp(NEG)); their output
regions are written once during prep from a constant band, hiding that DMA
under setup compute. Each label's remaining output goes out as a 2 MB
contiguous chunk (rows 0-511) plus a 1 MB strided chunk (rows 512-1023,
y >= 512), double-buffered so DMA, PE, DVE and ACT overlap.
"""

import numpy as np

B, S, I, H, O = 4, 1024, 768, 64, 16
NCORES = 8
OH = O // 2  # 8 labels per core
NEG = 1.0e12
KT = I // 128  # 6 k-tiles over the input dim
ST = S // 128  # 8 s-tiles

_STATE = {}


def _tables():
    """Host-precomputed constants (mimic reference fp32 ops)."""
    position = np.arange(S, dtype=np.float32)
    idx = np.arange(H // 2, dtype=np.float32)
    expo = (np.float32(-2.0) * idx) / np.float32(H)
    inv_freq = np.power(np.float32(10000.0), expo).astype(np.float32)
    ang = position[:, None] * inv_freq[None, :]          # [S, 32] f32
    cos_h = np.cos(ang).astype(np.float32).T             # [32, S]
    sin_h = np.sin(ang).astype(np.float32).T
    cosT = np.ascontiguousarray(np.concatenate([cos_h, cos_h], axis=0))  # [64, S]
    sinT = np.ascontiguousarray(np.concatenate([sin_h, sin_h], axis=0))
    # swap matrix as lhsT: out[2m] = -in[2m+1]; out[2m+1] = in[2m]
    msw = np.zeros((H, H), np.float32)
    for m in range(H // 2):
        msw[2 * m + 1, 2 * m] = -1.0
        msw[2 * m, 2 * m + 1] = 1.0
    # tril patterns for diagonal-crossing blocks: P_k[x', y'] = -NEG if x'+128k > y'
    xp = np.arange(128, dtype=np.int64)[:, None]
    yp = np.arange(512, dtype=np.int64)[None, :]
    tril = np.stack(
        [np.where(xp + 128 * k > yp, np.float32(-NEG), np.float32(0.0))
         for k in range(4)]
    ).astype(np.float32)                                  # [4, 128, 512]
    return cosT, sinT, msw, tril


def _build():
    import concourse.bacc as bacc
    import concourse.bass as bass
    import concourse.mybir as mybir
    from concourse import tile

    f32 = mybir.dt.float32
    f32r = mybir.dt.float32r
    AF = mybir.ActivationFunctionType
    ALU = mybir.AluOpType
    PSUM = bass.MemorySpace.PSUM

    nc = bacc.Bacc("TRN2", target_bir_lowering=False, debug=False,
                   num_devices=NCORES)

    xT_t = nc.dram_tensor("xT", [I, S], f32, kind="ExternalInput")
    mask_t = nc.dram_tensor("mask", [1, S], f32, kind="ExternalInput")
    ws_t = nc.dram_tensor("w_start", [I, H], f32, kind="ExternalInput")
    we_t = nc.dram_tensor("w_end", [I, H], f32, kind="ExternalInput")
    bs_t = nc.dram_tensor("b_start", [H, 1], f32, kind="ExternalInput")
    be_t = nc.dram_tensor("b_end", [H, 1], f32, kind="ExternalInput")
    wo_t = nc.dram_tensor("w_o", [OH, H, H], f32, kind="ExternalInput")
    cos_t = nc.dram_tensor("cos_t", [H, S], f32, kind="ExternalInput")
    sin_t = nc.dram_tensor("sin_t", [H, S], f32, kind="ExternalInput")
    msw_t = nc.dram_tensor("mswap", [H, H], f32, kind="ExternalInput")
    tril_t = nc.dram_tensor("trilneg", [4, 128, 512], f32, kind="ExternalInput")
    out_t = nc.dram_tensor("out", [OH, S, S], f32, kind="ExternalOutput")

    # [o, c, p, xb, y]: row = 512c + 128xb + p
    out_r = out_t.ap().rearrange("o (c xb p) y -> o c p xb y", c=2, xb=4, p=128)

    def r(ap):
        return ap.bitcast(f32r)

    with tile.TileContext(nc) as tc:
        with tc.tile_pool(name="persist", bufs=1) as pp, \
             tc.tile_pool(name="scratch", bufs=2) as sp:
            # ---- constants ----
            cosT = pp.tile([H, S], f32)
            nc.sync.dma_start(cosT[:], cos_t.ap())
            sinT = pp.tile([H, S], f32)
            nc.sync.dma_start(sinT[:], sin_t.ap())
            tril = pp.tile([128, 4, 512], f32)
            nc.sync.dma_start(tril[:], tril_t.ap().rearrange("k p y -> p k y"))
            bsv = pp.tile([H, 1], f32)
            nc.sync.dma_start(bsv[:], bs_t.ap())
            bev = pp.tile([H, 1], f32)
            nc.sync.dma_start(bev[:], be_t.ap())
            mask0 = pp.tile([1, S], f32)
            nc.sync.dma_start(mask0[:], mask_t.ap())
            ones1f = pp.tile([1, 128], f32)
            nc.gpsimd.memset(ones1f[:], 1.0)
            ones1 = pp.tile([1, 128], f32)
            nc.vector.tensor_copy(r(ones1[:]), ones1f[:])
            onesrow = pp.tile([1, S], f32)
            nc.gpsimd.memset(onesrow[:], 1.0)
            onesrowr = pp.tile([1, S], f32)
            nc.vector.tensor_copy(r(onesrowr[:]), onesrow[:])

            # matmul operands: DMA f32 then round to f32r on-chip
            wsT = pp.tile([128, KT, H], f32)
            weT = pp.tile([128, KT, H], f32)
            wo = pp.tile([H, OH, H], f32)
            mswap = pp.tile([H, H], f32)
            mask0r = pp.tile([1, S], f32)
            xTr = pp.tile([128, KT, S], f32)

            startR = pp.tile([H, S], f32)
            endA = pp.tile([H + 1, S], f32)       # 0..63 endR*pad, 64 addrow
            padB = pp.tile([H, S], f32)
            constband = pp.tile([128, 4, 512], f32)  # 4 copies of const row band
            addrow0 = pp.tile([1, S], f32)

            with tc.tile_pool(name="load", bufs=1) as lp:
                wsL = lp.tile([128, KT, H], f32)
                nc.sync.dma_start(
                    wsL[:], ws_t.ap().rearrange("(t p) h -> p t h", p=128))
                nc.vector.tensor_copy(r(wsT[:]), wsL[:])
                weL = lp.tile([128, KT, H], f32)
                nc.sync.dma_start(
                    weL[:], we_t.ap().rearrange("(t p) h -> p t h", p=128))
                nc.vector.tensor_copy(r(weT[:]), weL[:])
                woL = lp.tile([H, OH, H], f32)
                nc.sync.dma_start(woL[:], wo_t.ap().rearrange("o i j -> i o j"))
                nc.vector.tensor_copy(r(wo[:]), woL[:])
                mswL = lp.tile([H, H], f32)
                nc.sync.dma_start(mswL[:], msw_t.ap())
                nc.vector.tensor_copy(r(mswap[:]), mswL[:])
                nc.vector.tensor_copy(r(mask0r[:]), mask0[:])

                xTin = lp.tile([128, KT, S], f32)
                xg = xT_t.ap().rearrange("(t p) s -> p t s", p=128)
                for t in range(KT):
                    nc.sync.dma_start(xTin[:, t, :], xg[:, t, :])
                    if t % 2 == 0:
                        nc.vector.tensor_copy(r(xTr[:, t, :]), xTin[:, t, :])
                    else:
                        nc.scalar.copy(r(xTr[:, t, :]), xTin[:, t, :])

                # ---- mask-derived vectors ----
                nc.vector.tensor_scalar(
                    r(addrow0[:]), mask0[:], float(NEG), float(-NEG),
                    ALU.mult, ALU.add)                 # (pad-1)*NEG

                with tc.tile_pool(name="ps_proj", bufs=2, space=PSUM) as ppj:
                    # pad broadcast + constant band via K=1 fp32r matmuls
                    for h in range(2):
                        sl = slice(h * 512, (h + 1) * 512)
                        ps_pb = ppj.tile([H, 512], f32, name="ps_pb")
                        nc.tensor.matmul(ps_pb[:], r(ones1[:, :H]),
                                         r(mask0r[:, sl]),
                                         start=True, stop=True)
                        nc.scalar.copy(padB[:, sl], ps_pb[:])
                    ps_cb = ppj.tile([128, 512], f32, name="ps_cb", bufs=1)
                    nc.tensor.matmul(ps_cb[:], r(ones1[:]), r(addrow0[:, 0:512]),
                                     start=True, stop=True)
                    nc.scalar.activation(constband[:, 0, :], ps_cb[:], AF.Copy,
                                         bias=float(-NEG))
                    for j in range(1, 4):
                        nc.gpsimd.tensor_copy(constband[:, j, :],
                                              constband[:, 0, :])
                    # constant (below-diagonal) output regions for every label:
                    # rows 512..1023, y < 512 — write them now, while DMA is idle
                    for o in range(OH):
                        nc.sync.dma_start(out_r[o, 1][:, :, 0:512],
                                          constband[:])

                    # ---- projections + rotation ----
                    for side in range(2):             # 0 = start, 1 = end
                        wT = wsT if side == 0 else weT
                        bv = bsv if side == 0 else bev
                        for h in range(2):
                            sl = slice(h * 512, (h + 1) * 512)
                            ps_proj = ppj.tile([H, 512], f32, name="ps_proj")
                            for kb in range(KT):
                                nc.tensor.matmul(
                                    ps_proj[:], r(wT[:, kb, :]),
                                    r(xTr[:, kb, sl]),
                                    start=(kb == 0), stop=(kb == KT - 1))
                            relu = sp.tile([H, 512], f32, name="relu")
                            nc.scalar.activation(r(relu[:]), ps_proj[:],
                                                 AF.Relu, bias=bv[:])
                            ps_swap = ppj.tile([H, 512], f32, name="ps_swap")
                            nc.tensor.matmul(ps_swap[:], r(mswap[:]), r(relu[:]),
                                             start=True, stop=True)
                            rm = sp.tile([H, 512], f32, name="rm")
                            nc.vector.tensor_mul(rm[:], relu[:], cosT[:, sl])
                            rs = sp.tile([H, 512], f32, name="rs")
                            nc.vector.tensor_mul(rs[:], ps_swap[:], sinT[:, sl])
                            if side == 0:
                                nc.vector.tensor_add(r(startR[:, sl]),
                                                     rm[:], rs[:])
                            else:
                                es = sp.tile([H, 512], f32, name="es")
                                nc.vector.tensor_add(es[:], rm[:], rs[:])
                                nc.vector.tensor_mul(r(endA[0:H, sl]), es[:],
                                                     padB[:, sl])
                # row 64 of endA = addrow (cross-partition move via DMA)
                nc.sync.dma_start(r(endA[H:H + 1, :]), r(addrow0[:]))

            # ---- label double-buffers ----
            tmpA0 = pp.tile([H + 1, S], f32)
            tmpA1 = pp.tile([H + 1, S], f32)
            nc.sync.dma_start(r(tmpA0[H:H + 1, :]), r(onesrowr[:]))
            nc.sync.dma_start(r(tmpA1[H:H + 1, :]), r(onesrowr[:]))

            # ---- main loop over labels ----
            with tc.tile_pool(name="stg0_pool", bufs=2) as st0, \
                 tc.tile_pool(name="stg1_pool", bufs=2) as st1, \
                 tc.tile_pool(name="ps_main", bufs=2, space=PSUM) as pm, \
                 tc.tile_pool(name="ps_span_pool", bufs=5, space=PSUM) as pspan:
                for o in range(OH):
                    tmpA = tmpA0 if o % 2 == 0 else tmpA1
                    # tmpT[j, x] = sum_i weight[o][i, j] * startR[i, x]
                    for h in range(2):
                        sl = slice(h * 512, (h + 1) * 512)
                        ps_tmp = pm.tile([H, 512], f32, name="ps_tmp")
                        nc.tensor.matmul(ps_tmp[:],
                                         r(wo[:, o, :]), r(startR[:, sl]),
                                         start=True, stop=True)
                        nc.scalar.copy(r(tmpA[0:H, sl]), ps_tmp[:])
                    # chunk 0: rows 0-511 (xb 0-3), both y halves
                    stg0 = st0.tile([128, 4, S], f32, name="stg0")
                    for xb in range(4):
                        lhs = r(tmpA[:, xb * 128:(xb + 1) * 128])
                        ps_sp = pspan.tile([128, 512], f32, name="ps_sp",
                                           tag="ps_sp")
                        nc.tensor.matmul(ps_sp[:], lhs, r(endA[:, 0:512]),
                                         start=True, stop=True)
                        nc.vector.tensor_tensor(stg0[:, xb, 0:512], ps_sp[:],
                                                tril[:, xb, :], ALU.add)
                        ps_sp2 = pspan.tile([128, 512], f32, name="ps_sp2",
                                            tag="ps_sp")
                        nc.tensor.matmul(ps_sp2[:], lhs, r(endA[:, 512:1024]),
                                         start=True, stop=True)
                        nc.scalar.copy(stg0[:, xb, 512:1024], ps_sp2[:])
                    nc.sync.dma_start(out_r[o, 0], stg0[:])
                    # chunk 1: rows 512-1023 (xb 4-7), computed y half only
                    stg1 = st1.tile([128, 4, 512], f32, name="stg1")
                    for xb in range(4, 8):
                        lhs = r(tmpA[:, xb * 128:(xb + 1) * 128])
                        ps_sp3 = pspan.tile([128, 512], f32, name="ps_sp3",
                                            tag="ps_sp")
                        nc.tensor.matmul(ps_sp3[:], lhs, r(endA[:, 512:1024]),
                                         start=True, stop=True)
                        nc.vector.tensor_tensor(stg1[:, xb - 4, :], ps_sp3[:],
                                                tril[:, xb - 4, :], ALU.add)
                    nc.sync.dma_start(out_r[o, 1][:, :, 512:1024], stg1[:])

    nc.compile()
    return nc


def _get_nc():
    if "nc" not in _STATE:
        _STATE["nc"] = _build()
    return _STATE["nc"]


def _make_in_maps(x, mask, W_start, b_start, W_end, b_end, weight):
    cosT, sinT, msw, tril = _tables()
    x = np.asarray(x, np.float32)
    mask = np.ascontiguousarray(np.asarray(mask, np.float32))
    W_start = np.ascontiguousarray(np.asarray(W_start, np.float32))
    W_end = np.ascontiguousarray(np.asarray(W_end, np.float32))
    b_start = np.ascontiguousarray(np.asarray(b_start, np.float32).reshape(H, 1))
    b_end = np.ascontiguousarray(np.asarray(b_end, np.float32).reshape(H, 1))
    weight = np.ascontiguousarray(np.asarray(weight, np.float32))
    in_maps = []
    for c in range(NCORES):
        b, half = c // 2, c % 2
        in_maps.append({
            "xT": np.ascontiguousarray(x[b].T),
            "mask": np.ascontiguousarray(mask[b:b + 1]),
            "w_start": W_start,
            "w_end": W_end,
            "b_start": b_start,
            "b_end": b_end,
            "w_o": np.ascontiguousarray(weight[half * OH:(half + 1) * OH]),
            "cos_t": cosT,
            "sin_t": sinT,
            "mswap": msw,
            "trilneg": tril,
        })
    return in_maps


def _execute(in_maps, trace=False):
    from concourse.bass_utils import run_bass_kernel_spmd
    nc = _get_nc()
    return run_bass_kernel_spmd(nc, in_maps, list(range(NCORES)), trace=trace)


def kernel(x, mask, W_start, b_start, W_end, b_end, weight):
    in_maps = _make_in_maps(x, mask, W_start, b_start, W_end, b_end, weight)
    res = _execute(in_maps)
    outs = [res.results[c]["out"] for c in range(NCORES)]
    full = np.stack(outs).reshape(B, 2, OH, S, S).reshape(B, O, S, S)
    return full.astype(np.float32)


# revision 11
# speedup vs baseline: 2.2091x; 1.0539x over previous
"""Biaffine span classifier kernel for 8 Trainium2 NeuronCores.

Math (per batch b, label o):
    start = relu(x @ W_start + b_start); end = relu(x @ W_end + b_end)
    rotate both with tiled-halves sinusoidal tables
    span[o,x,y] = startR[x,:] @ weight[o] @ endR[y,:]^T
    span = span*pad[y] - (1-pad[y])*NEG - NEG*tril(x>y)

Sharding: core c = b*2 + half handles batch b and labels [half*8, half*8+8).
Each core writes a contiguous [8, S, S] slice of the output.

On-chip layout is transposed ([H, S], H on partitions); x is transposed on
the host so every contraction has its reduction dim on partitions. All
matmuls run in fp32r mode (single-pass PE, ~TF32 precision) — operands are
rounded to fp32r by their producers as the BIR verifier requires. The mask's
additive term is folded into the big matmul via an augmented K=65
contraction (ones row in tmpT, add_row in endT). Blocks entirely below the
diagonal are exactly -NEG in fp32 (|span| << 0.5*ulp(NEG)); their output
regions are written once during prep from a constant band, hiding that DMA
under setup compute. Each label's remaining output goes out as a 2 MB
contiguous chunk (rows 0-511) plus a 1 MB strided chunk (rows 512-1023,
y >= 512), double-buffered so DMA, PE, DVE and ACT overlap.
"""

import numpy as np

B, S, I, H, O = 4, 1024, 768, 64, 16
NCORES = 8
OH = O // 2  # 8 labels per core
NEG = 1.0e12
KT = I // 128  # 6 k-tiles over the input dim
ST = S // 128  # 8 s-tiles

_STATE = {}


def _tables():
    """Host-precomputed constants (mimic reference fp32 ops)."""
    position = np.arange(S, dtype=np.float32)
    idx = np.arange(H // 2, dtype=np.float32)
    expo = (np.float32(-2.0) * idx) / np.float32(H)
    inv_freq = np.power(np.float32(10000.0), expo).astype(np.float32)
    ang = position[:, None] * inv_freq[None, :]          # [S, 32] f32
    cos_h = np.cos(ang).astype(np.float32).T             # [32, S]
    sin_h = np.sin(ang).astype(np.float32).T
    cosT = np.ascontiguousarray(np.concatenate([cos_h, cos_h], axis=0))  # [64, S]
    sinT = np.ascontiguousarray(np.concatenate([sin_h, sin_h], axis=0))
    # swap matrix as lhsT: out[2m] = -in[2m+1]; out[2m+1] = in[2m]
    msw = np.zeros((H, H), np.float32)
    for m in range(H // 2):
        msw[2 * m + 1, 2 * m] = -1.0
        msw[2 * m, 2 * m + 1] = 1.0
    # tril patterns for diagonal-crossing blocks: P_k[x', y'] = -NEG if x'+128k > y'
    xp = np.arange(128, dtype=np.int64)[:, None]
    yp = np.arange(512, dtype=np.int64)[None, :]
    tril = np.stack(
        [np.where(xp + 128 * k > yp, np.float32(-NEG), np.float32(0.0))
         for k in range(4)]
    ).astype(np.float32)                                  # [4, 128, 512]
    return cosT, sinT, msw, tril


def _build():
    import concourse.bacc as bacc
    import concourse.bass as bass
    import concourse.mybir as mybir
    from concourse import tile

    f32 = mybir.dt.float32
    f32r = mybir.dt.float32r
    AF = mybir.ActivationFunctionType
    ALU = mybir.AluOpType
    PSUM = bass.MemorySpace.PSUM

    nc = bacc.Bacc("TRN2", target_bir_lowering=False, debug=False,
                   num_devices=NCORES)

    xT_t = nc.dram_tensor("xT", [I, S], f32, kind="ExternalInput")
    mask_t = nc.dram_tensor("mask", [1, S], f32, kind="ExternalInput")
    ws_t = nc.dram_tensor("w_start", [I, H], f32, kind="ExternalInput")
    we_t = nc.dram_tensor("w_end", [I, H], f32, kind="ExternalInput")
    bs_t = nc.dram_tensor("b_start", [H, 1], f32, kind="ExternalInput")
    be_t = nc.dram_tensor("b_end", [H, 1], f32, kind="ExternalInput")
    wo_t = nc.dram_tensor("w_o", [OH, H, H], f32, kind="ExternalInput")
    cos_t = nc.dram_tensor("cos_t", [H, S], f32, kind="ExternalInput")
    sin_t = nc.dram_tensor("sin_t", [H, S], f32, kind="ExternalInput")
    msw_t = nc.dram_tensor("mswap", [H, H], f32, kind="ExternalInput")
    tril_t = nc.dram_tensor("trilneg", [4, 128, 512], f32, kind="ExternalInput")
    out_t = nc.dram_tensor("out", [OH, S, S], f32, kind="ExternalOutput")

    # [o, c, p, xb, y]: row = 512c + 128xb + p
    out_r = out_t.ap().rearrange("o (c xb p) y -> o c p xb y", c=2, xb=4, p=128)

    def r(ap):
        return ap.bitcast(f32r)

    with tile.TileContext(nc) as tc:
        with tc.tile_pool(name="persist", bufs=1) as pp, \
             tc.tile_pool(name="scratch", bufs=2) as sp:
            # ---- constants ----
            cosT = pp.tile([H, S], f32)
            nc.sync.dma_start(cosT[:], cos_t.ap())
            sinT = pp.tile([H, S], f32)
            nc.sync.dma_start(sinT[:], sin_t.ap())
            tril = pp.tile([128, 4, 512], f32)
            nc.sync.dma_start(tril[:], tril_t.ap().rearrange("k p y -> p k y"))
            bsv = pp.tile([H, 1], f32)
            nc.sync.dma_start(bsv[:], bs_t.ap())
            bev = pp.tile([H, 1], f32)
            nc.sync.dma_start(bev[:], be_t.ap())
            mask0 = pp.tile([1, S], f32)
            nc.sync.dma_start(mask0[:], mask_t.ap())
            ones1f = pp.tile([1, 128], f32)
            nc.gpsimd.memset(ones1f[:], 1.0)
            ones1 = pp.tile([1, 128], f32)
            nc.vector.tensor_copy(r(ones1[:]), ones1f[:])
            onesrow = pp.tile([1, S], f32)
            nc.gpsimd.memset(onesrow[:], 1.0)
            onesrowr = pp.tile([1, S], f32)
            nc.vector.tensor_copy(r(onesrowr[:]), onesrow[:])

            # matmul operands: DMA f32 then round to f32r on-chip
            wsT = pp.tile([128, KT, H], f32)
            weT = pp.tile([128, KT, H], f32)
            wo = pp.tile([H, OH, H], f32)
            mswap = pp.tile([H, H], f32)
            mask0r = pp.tile([1, S], f32)
            xTr = pp.tile([128, KT, S], f32)

            startR = pp.tile([H, S], f32)
            endA = pp.tile([H + 1, S], f32)       # 0..63 endR*pad, 64 addrow
            padB = pp.tile([H, S], f32)
            constband = pp.tile([128, 4, 512], f32)  # 4 copies of const row band
            addrow0 = pp.tile([1, S], f32)

            with tc.tile_pool(name="load", bufs=1) as lp:
                wsL = lp.tile([128, KT, H], f32)
                nc.sync.dma_start(
                    wsL[:], ws_t.ap().rearrange("(t p) h -> p t h", p=128))
                nc.vector.tensor_copy(r(wsT[:]), wsL[:])
                weL = lp.tile([128, KT, H], f32)
                nc.sync.dma_start(
                    weL[:], we_t.ap().rearrange("(t p) h -> p t h", p=128))
                nc.vector.tensor_copy(r(weT[:]), weL[:])
                woL = lp.tile([H, OH, H], f32)
                nc.sync.dma_start(woL[:], wo_t.ap().rearrange("o i j -> i o j"))
                nc.vector.tensor_copy(r(wo[:]), woL[:])
                mswL = lp.tile([H, H], f32)
                nc.sync.dma_start(mswL[:], msw_t.ap())
                nc.vector.tensor_copy(r(mswap[:]), mswL[:])
                nc.vector.tensor_copy(r(mask0r[:]), mask0[:])

                xTin = lp.tile([128, KT, S], f32)
                xg = xT_t.ap().rearrange("(t p) s -> p t s", p=128)
                for t in range(KT):
                    nc.sync.dma_start(xTin[:, t, :], xg[:, t, :])
                    if t % 2 == 0:
                        nc.vector.tensor_copy(r(xTr[:, t, :]), xTin[:, t, :])
                    else:
                        nc.scalar.copy(r(xTr[:, t, :]), xTin[:, t, :])

                # ---- mask-derived vectors ----
                nc.vector.tensor_scalar(
                    r(addrow0[:]), mask0[:], float(NEG), float(-NEG),
                    ALU.mult, ALU.add)                 # (pad-1)*NEG

                with tc.tile_pool(name="ps_proj", bufs=2, space=PSUM) as ppj:
                    # pad broadcast + constant band via K=1 fp32r matmuls
                    for h in range(2):
                        sl = slice(h * 512, (h + 1) * 512)
                        ps_pb = ppj.tile([H, 512], f32, name="ps_pb")
                        nc.tensor.matmul(ps_pb[:], r(ones1[:, :H]),
                                         r(mask0r[:, sl]),
                                         start=True, stop=True)
                        nc.scalar.copy(padB[:, sl], ps_pb[:])
                    ps_cb = ppj.tile([128, 512], f32, name="ps_cb", bufs=1)
                    nc.tensor.matmul(ps_cb[:], r(ones1[:]), r(addrow0[:, 0:512]),
                                     start=True, stop=True)
                    nc.scalar.activation(constband[:, 0, :], ps_cb[:], AF.Copy,
                                         bias=float(-NEG))
                    for j in range(1, 4):
                        nc.gpsimd.tensor_copy(constband[:, j, :],
                                              constband[:, 0, :])
                    # constant (below-diagonal) output regions for every label:
                    # rows 512..1023, y < 512 — write them now, while DMA is
                    # otherwise idle. SWDGE queue so they can't block the small
                    # HWDGE moves that gate the first span matmuls.
                    for o in range(OH):
                        nc.gpsimd.dma_start(out_r[o, 1][:, :, 0:512],
                                            constband[:])

                    # ---- projections + rotation ----
                    for side in range(2):             # 0 = start, 1 = end
                        wT = wsT if side == 0 else weT
                        bv = bsv if side == 0 else bev
                        for h in range(2):
                            sl = slice(h * 512, (h + 1) * 512)
                            ps_proj = ppj.tile([H, 512], f32, name="ps_proj")
                            for kb in range(KT):
                                nc.tensor.matmul(
                                    ps_proj[:], r(wT[:, kb, :]),
                                    r(xTr[:, kb, sl]),
                                    start=(kb == 0), stop=(kb == KT - 1))
                            relu = sp.tile([H, 512], f32, name="relu")
                            nc.scalar.activation(r(relu[:]), ps_proj[:],
                                                 AF.Relu, bias=bv[:])
                            ps_swap = ppj.tile([H, 512], f32, name="ps_swap")
                            nc.tensor.matmul(ps_swap[:], r(mswap[:]), r(relu[:]),
                                             start=True, stop=True)
                            rm = sp.tile([H, 512], f32, name="rm")
                            nc.vector.tensor_mul(rm[:], relu[:], cosT[:, sl])
                            rs = sp.tile([H, 512], f32, name="rs")
                            nc.vector.tensor_mul(rs[:], ps_swap[:], sinT[:, sl])
                            if side == 0:
                                nc.vector.tensor_add(r(startR[:, sl]),
                                                     rm[:], rs[:])
                            else:
                                es = sp.tile([H, 512], f32, name="es")
                                nc.vector.tensor_add(es[:], rm[:], rs[:])
                                nc.vector.tensor_mul(r(endA[0:H, sl]), es[:],
                                                     padB[:, sl])
                # row 64 of endA = addrow (cross-partition move via DMA)
                nc.scalar.dma_start(r(endA[H:H + 1, :]), r(addrow0[:]))

            # ---- label double-buffers ----
            tmpA0 = pp.tile([H + 1, S], f32)
            tmpA1 = pp.tile([H + 1, S], f32)
            nc.scalar.dma_start(r(tmpA0[H:H + 1, :]), r(onesrowr[:]))
            nc.scalar.dma_start(r(tmpA1[H:H + 1, :]), r(onesrowr[:]))

            # ---- main loop over labels ----
            with tc.tile_pool(name="stg0_pool", bufs=2) as st0, \
                 tc.tile_pool(name="stg1_pool", bufs=2) as st1, \
                 tc.tile_pool(name="ps_main", bufs=2, space=PSUM) as pm, \
                 tc.tile_pool(name="ps_span_pool", bufs=5, space=PSUM) as pspan:
                for o in range(OH):
                    tmpA = tmpA0 if o % 2 == 0 else tmpA1
                    # tmpT[j, x] = sum_i weight[o][i, j] * startR[i, x]
                    for h in range(2):
                        sl = slice(h * 512, (h + 1) * 512)
                        ps_tmp = pm.tile([H, 512], f32, name="ps_tmp")
                        nc.tensor.matmul(ps_tmp[:],
                                         r(wo[:, o, :]), r(startR[:, sl]),
                                         start=True, stop=True)
                        nc.scalar.copy(r(tmpA[0:H, sl]), ps_tmp[:])
                    # chunk 0: rows 0-511 (xb 0-3), both y halves
                    stg0 = st0.tile([128, 4, S], f32, name="stg0")
                    for xb in range(4):
                        lhs = r(tmpA[:, xb * 128:(xb + 1) * 128])
                        ps_sp = pspan.tile([128, 512], f32, name="ps_sp",
                                           tag="ps_sp")
                        nc.tensor.matmul(ps_sp[:], lhs, r(endA[:, 0:512]),
                                         start=True, stop=True)
                        nc.vector.tensor_tensor(stg0[:, xb, 0:512], ps_sp[:],
                                                tril[:, xb, :], ALU.add)
                        ps_sp2 = pspan.tile([128, 512], f32, name="ps_sp2",
                                            tag="ps_sp")
                        nc.tensor.matmul(ps_sp2[:], lhs, r(endA[:, 512:1024]),
                                         start=True, stop=True)
                        nc.scalar.copy(stg0[:, xb, 512:1024], ps_sp2[:])
                    nc.sync.dma_start(out_r[o, 0], stg0[:])
                    # chunk 1: rows 512-1023 (xb 4-7), computed y half only
                    stg1 = st1.tile([128, 4, 512], f32, name="stg1")
                    for xb in range(4, 8):
                        lhs = r(tmpA[:, xb * 128:(xb + 1) * 128])
                        ps_sp3 = pspan.tile([128, 512], f32, name="ps_sp3",
                                            tag="ps_sp")
                        nc.tensor.matmul(ps_sp3[:], lhs, r(endA[:, 512:1024]),
                                         start=True, stop=True)
                        nc.vector.tensor_tensor(stg1[:, xb - 4, :], ps_sp3[:],
                                                tril[:, xb - 4, :], ALU.add)
                    nc.sync.dma_start(out_r[o, 1][:, :, 512:1024], stg1[:])

    nc.compile()
    return nc


def _get_nc():
    if "nc" not in _STATE:
        _STATE["nc"] = _build()
    return _STATE["nc"]


def _make_in_maps(x, mask, W_start, b_start, W_end, b_end, weight):
    cosT, sinT, msw, tril = _tables()
    x = np.asarray(x, np.float32)
    mask = np.ascontiguousarray(np.asarray(mask, np.float32))
    W_start = np.ascontiguousarray(np.asarray(W_start, np.float32))
    W_end = np.ascontiguousarray(np.asarray(W_end, np.float32))
    b_start = np.ascontiguousarray(np.asarray(b_start, np.float32).reshape(H, 1))
    b_end = np.ascontiguousarray(np.asarray(b_end, np.float32).reshape(H, 1))
    weight = np.ascontiguousarray(np.asarray(weight, np.float32))
    in_maps = []
    for c in range(NCORES):
        b, half = c // 2, c % 2
        in_maps.append({
            "xT": np.ascontiguousarray(x[b].T),
            "mask": np.ascontiguousarray(mask[b:b + 1]),
            "w_start": W_start,
            "w_end": W_end,
            "b_start": b_start,
            "b_end": b_end,
            "w_o": np.ascontiguousarray(weight[half * OH:(half + 1) * OH]),
            "cos_t": cosT,
            "sin_t": sinT,
            "mswap": msw,
            "trilneg": tril,
        })
    return in_maps


def _execute(in_maps, trace=False):
    from concourse.bass_utils import run_bass_kernel_spmd
    nc = _get_nc()
    return run_bass_kernel_spmd(nc, in_maps, list(range(NCORES)), trace=trace)


def kernel(x, mask, W_start, b_start, W_end, b_end, weight):
    in_maps = _make_in_maps(x, mask, W_start, b_start, W_end, b_end, weight)
    res = _execute(in_maps)
    outs = [res.results[c]["out"] for c in range(NCORES)]
    full = np.stack(outs).reshape(B, 2, OH, S, S).reshape(B, O, S, S)
    return full.astype(np.float32)
